# revision 44
# baseline (speedup 1.0000x reference)
"""Causal self-attention (B=4, T=2048, H=8, hd=128, D=1024) on 8 trn2 cores.

Sharding: core c handles batch b = c//2, head-group g = c%2 (heads 4g..4g+4).
Each core computes qkv projection for its 4 heads, rms-norm + rope on q/k,
v = l0*v + l1*ve, causal attention, and a partial c_proj ([T, D]) over its
head group.  Host sums the two head-group partials per batch.

All matmuls run in bf16 (PSUM accumulation fp32).  lambda scalars are folded
into the weights / ve on the host.  The softmax normalizer path stays fp32
(fp32r broadcast matmul).
"""
import sys

sys.path.insert(0, "/opt/trn_rl_repo")

import numpy as np
import ml_dtypes

import concourse.bass as bass
import concourse.mybir as mybir
import concourse.tile as tile
from concourse import bacc
from concourse.bass import ts
from concourse.bass_utils import run_bass_kernel_spmd

F32 = mybir.dt.float32
F32R = mybir.dt.float32r
BF16 = mybir.dt.bfloat16
MULT = mybir.AluOpType.mult
ADD = mybir.AluOpType.add
AF = mybir.ActivationFunctionType

# ---- problem constants (hardcoded per the contract) ----
B, T, D = 4, 2048, 1024
H, HD = 8, 128
HG = 4          # heads per group
EG = HG * HD    # 512 cols per head-group
ATTN_SCALE = 0.12
RMS_EPS = 1.1920929e-07
P = 128
NT = T // P     # 16 t-blocks
ND = D // P     # 8 d-chunks
NW = T // 512   # 4 query windows
S2 = ATTN_SCALE * ATTN_SCALE

_CACHED = {}


def _rope_tables():
    af = (1.0 / 1024.0) ** np.linspace(0.0, 1.0, HD // 4, dtype=np.float32)
    af = np.concatenate([af, np.zeros(HD // 4, dtype=np.float32)])
    t = np.arange(T, dtype=np.float32)
    theta = np.einsum("i,j->ij", t, af)  # [T, 64]
    cos, sin = np.cos(theta), np.sin(theta)
    # cc: [c|c] per head; ss: [s|-s] per head -> [T, 4, 128] -> [T, 512]
    cc1 = np.concatenate([cos, cos], axis=1)            # [T,128]
    ss1 = np.concatenate([sin, -sin], axis=1)           # [T,128]
    cc = np.tile(cc1, (1, HG)).astype(np.float32)       # [T,512]
    ss = np.tile(ss1, (1, HG)).astype(np.float32)       # [T,512]
    return cc, ss


def _masks():
    # tri[tk, c] = 1 if c >= tk  (the causal edge band)
    tk = np.arange(P)[:, None]
    c = np.arange(P)[None, :]
    return (c >= tk).astype(np.float32)


def build(cfg=None):
    cfg = cfg or {}
    ptp_bufs = cfg.get("ptp_bufs", 6)
    warmup = cfg.get("warmup", 40)
    delay_rs = cfg.get("delay_rs", 3)
    nc = bacc.Bacc("TRN2", target_bir_lowering=False, debug=False)

    xT = nc.dram_tensor("xT", [D, T], BF16, kind="ExternalInput")
    wqkT = nc.dram_tensor("wqkT", [D, 2 * EG], BF16, kind="ExternalInput")
    wvT = nc.dram_tensor("wvT", [D, EG], BF16, kind="ExternalInput")
    ve = nc.dram_tensor("ve", [T, EG], BF16, kind="ExternalInput")
    cpT = nc.dram_tensor("cpT", [EG, D], BF16, kind="ExternalInput")
    cc_t = nc.dram_tensor("cc", [T, EG], BF16, kind="ExternalInput")
    ss_t = nc.dram_tensor("ss", [T, EG], BF16, kind="ExternalInput")
    mk_t = nc.dram_tensor("mk", [P, P], BF16, kind="ExternalInput")
    id_t = nc.dram_tensor("idm", [P, P], BF16, kind="ExternalInput")
    out = nc.dram_tensor("out", [T, D], F32, kind="ExternalOutput")

    xTr = xT.rearrange("(c p) t -> c p t", p=P)       # [8, 128, 2048]
    wqkr = wqkT.rearrange("(c p) e -> c p e", p=P)    # [8, 128, 1024]
    wvr = wvT.rearrange("(c p) e -> c p e", p=P)      # [8, 128, 512]
    cpr = cpT.rearrange("(c p) d -> c p d", p=P)      # [4, 128, 1024]
    ver = ve.rearrange("(i p) e -> i p e", p=P)       # [16, 128, 512]
    ccr = cc_t.rearrange("(i p) e -> i p e", p=P)
    ssr = ss_t.rearrange("(i p) e -> i p e", p=P)

    with tile.TileContext(nc) as tc:
        with (
            tc.tile_pool(name="persist", bufs=1) as pp,
            tc.tile_pool(name="consts", bufs=1) as cp,
        ):
            # persistent tensors ([P, HG, T] so one copy evicts all 4 heads)
            QTa = pp.tile([P, HG, T], BF16, tag="QTa", name="QTa")
            KTa = pp.tile([P, HG, T], BF16, tag="KTa", name="KTa")
            V = [pp.tile([P, EG], BF16, tag=f"V{i}", name=f"V{i}") for i in range(NT)]
            tri = cp.tile([P, P], BF16, tag="tri")
            SK = cp.tile([P, NT, HG], F32, tag="SK")
            ident = cp.tile([P, P], BF16, tag="ident")
            ones_col = cp.tile([P, 1], BF16, tag="ones_col")
            ones_row = cp.tile([1, P], F32, tag="ones_row")
            bias_q = cp.tile([P, 1], F32, tag="bias_q")
            bias_k = cp.tile([P, 1], F32, tag="bias_k")
            nc.vector.memset(bias_q[:], RMS_EPS / S2)
            nc.vector.memset(bias_k[:], float(RMS_EPS))
            nc.sync.dma_start(tri[:], mk_t[:, :])
            nc.sync.dma_start(ident[:], id_t[:, :])
            nc.vector.memset(ones_col[:], 1.0)
            nc.vector.memset(ones_row[:], 1.0)
            # c_proj weights prefetched during phase A
            cpt = [cp.tile([P, D], BF16, tag=f"cpt{e}", name=f"cpt{e}")
                   for e in range(HG)]
            for e in range(HG):
                nc.sync.dma_start(cpt[e][:], cpr[e])

            # ---------------- Phase A: projections, rms+rope, transposes ---
            with (
                tc.tile_pool(name="wpool", bufs=1) as wp,
                tc.tile_pool(name="xpool", bufs=3) as xp,
                tc.tile_pool(name="qkte", bufs=2) as qp,
                tc.tile_pool(name="ropetmp", bufs=2) as rp,
                tc.tile_pool(name="rrpool", bufs=5) as rrp,
                tc.tile_pool(name="tabs", bufs=2) as tp,
                tc.tile_pool(name="pA", bufs=2, space="PSUM") as pA,
                tc.tile_pool(name="pT", bufs=1, space="PSUM") as pT,
            ):
                xtis = {}

                def fetch_x(i):
                    if i < NT:
                        xi = xp.tile([P, ND, P], BF16, tag="xt", name="xt")
                        nc.sync.dma_start(
                            xi[:], xTr[:, :, ts(i, P)].rearrange("c p t -> p c t"))
                        xtis[i] = xi

                fetch_x(0)
                fetch_x(1)
                fetch_x(2)
                wqk = [wp.tile([P, 2 * EG], BF16, tag=f"wqk{c}", name=f"wqk{c}") for c in range(ND)]
                wv = [wp.tile([P, EG], BF16, tag=f"wv{c}", name=f"wv{c}") for c in range(ND)]
                for c in range(ND):
                    nc.sync.dma_start(wqk[c][:], wqkr[c])
                    nc.sync.dma_start(wv[c][:], wvr[c])

                if warmup:
                    wt = wp.tile([P, EG], BF16, tag="warmsrc", name="warmsrc")
                    nc.vector.memset(wt[:], 0.0)
                    for wi in range(warmup):
                        pw = pA.tile([P, EG], F32, tag="psq", name="warm")
                        nc.tensor.matmul(pw[0:1, :], ones_col[:], wt[:],
                                         start=True, stop=True)

                pendQ = []   # (fin, tsl) delayed 1 block
                pendK = []   # (rr_k, tsl) delayed 2 blocks

                def emit_q_transposes(rr_q, dg, tsl):
                    # regular matmul with moving = diag(rsc_h): computes
                    # rr^T @ diag(rsc) — transpose + per-token rms scale in one.
                    ptr = pT.tile([P, HG, P], F32, tag="ptrq", name="ptrq")
                    for h in range(HG):
                        nc.tensor.matmul(ptr[:, h, :], rr_q[:, ts(h, HD)],
                                         dg[:, h, :], start=True, stop=True)
                    nc.scalar.copy(QTa[:, :, tsl], ptr[:])

                def emit_k_transposes(rr_k, tsl):
                    ptr = pT.tile([P, HG, P], BF16, tag="ptrk", name="ptrk")
                    for h in range(HG):
                        nc.tensor.transpose(ptr[:, h, :], rr_k[:, ts(h, HD)],
                                            ident[:])
                    nc.scalar.copy(KTa[:, :, tsl], ptr[:])

                for i in range(NT):
                    tsl = ts(i, P)
                    xti = xtis.pop(i)

                    psq = pA.tile([P, EG], F32, tag="psq")
                    psk = pA.tile([P, EG], F32, tag="psk")
                    psv = pA.tile([P, EG], F32, tag="psv")
                    fetch_x(i + 3)
                    for c in range(ND):
                        nc.tensor.matmul(psq[:], xti[:, c, :], wqk[c][:, 0:EG],
                                         start=(c == 0), stop=(c == ND - 1))
                        nc.tensor.matmul(psk[:], xti[:, c, :], wqk[c][:, EG:2 * EG],
                                         start=(c == 0), stop=(c == ND - 1))
                        nc.tensor.matmul(psv[:], xti[:, c, :], wv[c][:],
                                         start=(c == 0), stop=(c == ND - 1))

                    # --- evict q/k early (frees PSUM for the next blocks) ----
                    qte = qp.tile([P, 2 * EG], BF16, tag="qte")
                    nc.scalar.copy(qte[:, EG:2 * EG], psk[:])
                    nc.scalar.copy(qte[:, 0:EG], psq[:])

                    cct = tp.tile([P, EG], BF16, tag="cct")
                    sst = tp.tile([P, EG], BF16, tag="sst")
                    nc.sync.dma_start(cct[:], ccr[i])
                    nc.sync.dma_start(sst[:], ssr[i])
                    s4 = sst[:].rearrange("p (h s e) -> p h s e", h=HG, s=2)

                    def rope_side(eng, src_ap, tag):
                        # t1 and the final add are contiguous [P, EG] ops; only
                        # the half-swap mults need the strided 4D view.
                        x4 = src_ap.rearrange("p (h s e) -> p h s e", h=HG, s=2)
                        t1 = rp.tile([P, EG], BF16, tag=f"t1_{tag}",
                                     name=f"t1_{tag}")
                        t2 = rp.tile([P, HG, 2, 64], BF16, tag=f"t2_{tag}",
                                     name=f"t2_{tag}")
                        eng.tensor_tensor(t1[:], src_ap, cct[:], op=MULT)
                        eng.tensor_tensor(t2[:, :, 0, :], x4[:, :, 1, :],
                                          s4[:, :, 0, :], op=MULT)
                        eng.tensor_tensor(t2[:, :, 1, :], x4[:, :, 0, :],
                                          s4[:, :, 1, :], op=MULT)
                        rr = rrp.tile([P, EG], BF16, tag=f"rr_{tag}",
                                      name=f"rr_{tag}")
                        t2f = t2[:].rearrange("p h s e -> p (h s e)")
                        eng.tensor_tensor(rr[:], t1[:], t2f, op=ADD)
                        return rr

                    # last blocks' k-rope on DVE: its chain gates phase B start
                    k_eng = nc.vector if i >= NT - 2 else nc.gpsimd
                    rr_k = rope_side(k_eng, qte[:, EG:2 * EG], "k")

                    # --- v = (l0*wv)x + (l1*ve)  (lambdas folded on host) ---
                    vet = tp.tile([P, EG], BF16, tag="vet")
                    nc.sync.dma_start(vet[:], ver[i])
                    nc.vector.tensor_tensor(V[i][:], psv[:], vet[:], op=ADD)

                    # --- q rms sumsq on DVE, then rope, then per-head scale --
                    ssq = rp.tile([P, 8], F32, tag="ssq")
                    sq_scr = rp.tile([P, P], F32, tag="sq_scr")
                    for h in range(4):
                        nc.vector.scalar_tensor_tensor(
                            sq_scr[:], qte[:, ts(h, HD)], 1.0, qte[:, ts(h, HD)],
                            op0=MULT, op1=MULT, accum_out=ssq[:, h:h + 1])
                    # scales: q gets 0.12 folded in; k scale is folded into the
                    # phase-B exp (per-partition scale), so only store recip.
                    sc = rp.tile([P, 8], F32, tag="sc")
                    nc.scalar.activation(sc[:, 0:4], ssq[:, 0:4], AF.Sqrt,
                                         scale=1.0 / (HD * S2), bias=bias_q[:])
                    rsc = rp.tile([P, 4], F32, tag="rsc")
                    nc.vector.reciprocal(rsc[:], sc[:, 0:4])
                    # diag(rsc_h) tiles for the scaled q-transposes; depends
                    # only on rsc so it overlaps the rope chain.
                    dg = rrp.tile([P, HG, P], BF16, tag="dg", name="dg")
                    for h in range(HG):
                        nc.vector.tensor_scalar(dg[:, h, :], ident[:],
                                                rsc[:, h:h + 1], None, op0=MULT)
                    rr_q = rope_side(nc.vector, qte[:, 0:EG], "q")

                    # --- k rms sumsq (only feeds phase-B exp scale) ----------
                    for h in range(4, 8):
                        nc.vector.scalar_tensor_tensor(
                            sq_scr[:], qte[:, ts(h, HD)], 1.0, qte[:, ts(h, HD)],
                            op0=MULT, op1=MULT, accum_out=ssq[:, h:h + 1])
                    nc.scalar.activation(sc[:, 4:8], ssq[:, 4:8], AF.Sqrt,
                                         scale=1.0 / HD, bias=bias_k[:])
                    nc.vector.reciprocal(SK[:, i, :], sc[:, 4:8])

                    # transposes of PREVIOUS blocks go after this block's
                    # projections in the PE queue (hides the elementwise
                    # chain); k is delayed 2 blocks (gpsimd rope is slow).
                    pendQ.append((rr_q, dg, tsl))
                    pendK.append((rr_k, tsl))
                    if len(pendQ) > 2:
                        emit_q_transposes(*pendQ.pop(0))
                    if len(pendK) > 3:
                        emit_k_transposes(*pendK.pop(0))
                for a in pendQ:
                    emit_q_transposes(*a)
                for a in pendK:
                    emit_k_transposes(*a)

            # ---------------- Phase B: attention + c_proj -------------------
            with (
                tc.tile_pool(name="ptpool", bufs=ptp_bufs) as ptp,
                tc.tile_pool(name="ypool", bufs=2) as yp,
                tc.tile_pool(name="rpool", bufs=2) as rpl,
                tc.tile_pool(name="opool", bufs=3) as op_,
                tc.tile_pool(name="pS", bufs=2, space="PSUM") as pS,
                tc.tile_pool(name="pY", bufs=2, space="PSUM") as pY,
                tc.tile_pool(name="pR", bufs=1, space="PSUM") as pR,
                tc.tile_pool(name="pO", bufs=2, space="PSUM") as pO,
            ):
                Ywin = {}       # w -> list of normalized Y tiles
                pending = None  # (w, h, ps_y, ps_r) awaiting normalization

                def emit_norm(pw, ph, ps_y, ps_r):
                    rro = rpl.tile([1, 512], F32, tag="rro", name="rro")
                    nc.vector.reciprocal_approx_fast(rro[:], ps_r[:])
                    bb = rpl.tile([P, 512], F32, tag="bb", name="bb")
                    rrow = rpl.tile([1, 512], F32R, tag="rrow", name="rrow")
                    nc.scalar.copy(rrow[:], rro[:])
                    ps_b = pR.tile([P, 512], F32, tag="ps_b", name="ps_b")
                    nc.tensor.matmul(ps_b[:], ones_row[:].bitcast(F32R),
                                     rrow[:], start=True, stop=True)
                    nc.vector.tensor_copy(bb[:], ps_b[:])
                    yh = yp.tile([P, 512], BF16, tag=f"y{ph}", name=f"y{ph}")
                    nc.vector.tensor_tensor(yh[:], ps_y[:], bb[:], op=MULT)
                    Ywin.setdefault(pw, []).append(yh)

                def emit_cproj(pw):
                    Y = Ywin.pop(pw)
                    for tb in range(4):
                        for db in range(2):
                            ps_o = pO.tile([P, 512], F32, tag="ps_o", name="ps_o")
                            for h in range(HG):
                                nc.tensor.matmul(ps_o[:], Y[h][:, ts(tb, P)],
                                                 cpt[h][:, ts(db, 512)],
                                                 start=(h == 0), stop=(h == HG - 1))
                            oc = op_.tile([P, 512], F32, tag="oc", name="oc")
                            nc.vector.tensor_copy(oc[:], ps_o[:])
                            nc.sync.dma_start(
                                out[pw * 512 + tb * P:pw * 512 + (tb + 1) * P,
                                    ts(db, 512)], oc[:])

                for w in range(NW):
                    wsl = ts(w, 512)
                    for h in range(HG):
                        nch = 4 * (w + 1)
                        ps_y = pY.tile([P, 512], F32, tag="ps_y", name="ps_y")
                        ps_r = pR.tile([1, 512], F32, tag="ps_r", name="ps_r")
                        rs_q = []

                        def v0_of(jj):
                            return max(jj - (nch - 4), 0) * P

                        def emit_pv(jj):
                            vv = v0_of(jj)
                            nc.tensor.matmul(ps_y[:, vv:512],
                                             V[jj][:, ts(h, HD)],
                                             rs_q[jj][:, vv:512],
                                             start=(jj == 0),
                                             stop=(jj == nch - 1))

                        # pv trails the score/exp stream by delay_rs j-steps
                        # so the PE never waits on the ACT exp of the same j.
                        # Row sums: pt pairs are pre-added on DVE, halving the
                        # PE row-sum matmuls (masked prefixes are zeroed).
                        prs = []

                        def emit_pair(m):
                            pa = rpl.tile([P, 512], BF16, tag="pa", name="pa")
                            va = v0_of(2 * m)
                            if va > 0:
                                nc.vector.memset(pa[:, 0:va], 0.0)
                            nc.vector.tensor_tensor(
                                pa[:, va:512], rs_q[2 * m][:, va:512],
                                rs_q[2 * m + 1][:, va:512], op=ADD)
                            prs.append(pa)

                        def emit_prs(m):
                            va = v0_of(2 * m)
                            nc.tensor.matmul(ps_r[:, va:512], ones_col[:],
                                             prs[m][:, va:512], start=(m == 0),
                                             stop=(m == nch // 2 - 1))

                        for j in range(nch):
                            v0 = v0_of(j)
                            ps_s = pS.tile([P, 512], F32, tag="ps_s", name="ps_s")
                            nc.tensor.matmul(ps_s[:, v0:512], KTa[:, h, ts(j, P)],
                                             QTa[:, h, w * 512 + v0:(w + 1) * 512],
                                             start=True, stop=True)
                            pt = ptp.tile([P, 512], BF16, tag="pt", name="pt")
                            if v0 > 0:
                                nc.vector.memset(pt[:, 0:v0], 0.0)
                            nc.scalar.activation(pt[:, v0:512], ps_s[:, v0:512],
                                                 AF.Exp, scale=SK[:, j, h:h + 1])
                            if j >= nch - 4:  # mask the 128-wide triangle band
                                nc.gpsimd.tensor_tensor(
                                    pt[:, v0:v0 + P], pt[:, v0:v0 + P], tri[:],
                                    op=MULT)
                            rs_q.append(pt)
                            if j % 2 == 1:
                                emit_pair(j // 2)
                                if j // 2 >= 1:
                                    emit_prs(j // 2 - 1)
                            if j >= delay_rs:
                                emit_pv(j - delay_rs)
                        for jj in range(max(nch - delay_rs, 0), nch):
                            emit_pv(jj)
                        emit_prs(nch // 2 - 1)
                        # normalize the PREVIOUS head now that this head's
                        # matmuls are queued (hides the recip->bcast latency)
                        if pending is not None:
                            emit_norm(*pending)
                            if pending[1] == HG - 1:
                                emit_cproj(pending[0])
                        pending = (w, h, ps_y, ps_r)
                emit_norm(*pending)
                emit_cproj(pending[0])
    nc.compile()
    return nc


def _get_nc():
    if "nc" not in _CACHED:
        _CACHED["nc"] = build()
    return _CACHED["nc"]


def _try_install_profile_shim():
    try:
        import contextlib
        import ctypes
        import types

        if "antenv.axon_hooks" in sys.modules:
            return
        so_path = "/opt/axon/libaxon_pjrt.so"
        lib = ctypes.CDLL(so_path)
        if not hasattr(lib, "axon_start_nrt_profile"):
            return
        lib.axon_start_nrt_profile.argtypes = [ctypes.POINTER(ctypes.c_int64),
                                               ctypes.c_size_t]
        lib.axon_start_nrt_profile.restype = ctypes.c_int64
        lib.axon_stop_nrt_profile.argtypes = [ctypes.c_char_p]
        lib.axon_stop_nrt_profile.restype = ctypes.c_int64

        @contextlib.contextmanager
        def _hook(output_dir, device_ids):
            import jax

            jax.devices()
            if device_ids:
                ids = (ctypes.c_int64 * len(device_ids))(*device_ids)
                rc = lib.axon_start_nrt_profile(ids, len(device_ids))
            else:
                rc = lib.axon_start_nrt_profile(None, 0)
            if rc != 0:
                raise RuntimeError(f"axon_start_nrt_profile rc={rc}")
            try:
                yield
            finally:
                lib.axon_stop_nrt_profile(str(output_dir).encode())

        mod = types.ModuleType("antenv.axon_hooks")
        mod.set_axon_ntff_profile_hook = lambda h: None
        mod.get_axon_ntff_profile_hook = lambda: _hook
        import antenv

        antenv.axon_hooks = mod
        sys.modules["antenv.axon_hooks"] = mod
    except Exception:
        pass


LAST_EXEC_TIME_NS = None


def kernel(x, ve, sa_lambdas, qkv_w, c_proj_weight):
    global LAST_EXEC_TIME_NS
    x = np.asarray(x, dtype=np.float32)
    ve = np.asarray(ve, dtype=np.float32)
    sa_lambdas = np.asarray(sa_lambdas, dtype=np.float32)
    qkv_w = np.asarray(qkv_w, dtype=np.float32)
    c_proj_weight = np.asarray(c_proj_weight, dtype=np.float32)

    def tobf(a):
        return np.ascontiguousarray(a).astype(ml_dtypes.bfloat16)

    cc, ss = _rope_tables()
    mk = _masks()
    idm = np.eye(P, dtype=np.float32)
    l0, l1 = float(sa_lambdas[0]), float(sa_lambdas[1])

    in_maps = []
    for c in range(8):
        b, g = c // 2, c % 2
        gs, ge = g * EG, (g + 1) * EG
        wq = qkv_w[0, gs:ge, :]           # [512, 1024]
        wk = qkv_w[1, gs:ge, :]
        wv = qkv_w[2, gs:ge, :] * l0      # lambda0 folded
        in_maps.append({
            "xT": tobf(x[b].T),                                       # [D, T]
            "wqkT": tobf(np.concatenate([wq, wk], axis=0).T),         # [D, 1024]
            "wvT": tobf(wv.T),                                        # [D, 512]
            "ve": tobf(
                ve[b].reshape(T, H, HD)[:, g * HG:(g + 1) * HG, :]
                .reshape(T, EG) * l1),                                # [T, 512]
            "cpT": tobf(c_proj_weight[:, gs:ge].T),                   # [512, D]
            "cc": tobf(cc), "ss": tobf(ss), "mk": tobf(mk),
            "idm": tobf(idm),
        })

    _try_install_profile_shim()
    nc = _get_nc()
    res = run_bass_kernel_spmd(nc, in_maps, core_ids=list(range(8)), trace=True)
    LAST_EXEC_TIME_NS = res.exec_time_ns

    outs = [res.results[c]["out"] for c in range(8)]
    full = np.stack([outs[2 * b] + outs[2 * b + 1] for b in range(B)], axis=0)
    return full.astype(np.float32)


# revision 45
# speedup vs baseline: 1.0051x; 1.0051x over previous
"""Causal self-attention (B=4, T=2048, H=8, hd=128, D=1024) on 8 trn2 cores.

Sharding: core c handles batch b = c//2, head-group g = c%2 (heads 4g..4g+4).
Each core computes qkv projection for its 4 heads, rms-norm + rope on q/k,
v = l0*v + l1*ve, causal attention, and a partial c_proj ([T, D]) over its
head group.  Host sums the two head-group partials per batch.

All matmuls run in bf16 (PSUM accumulation fp32).  lambda scalars are folded
into the weights / ve on the host.  The softmax normalizer path stays fp32
(fp32r broadcast matmul).
"""
import sys

sys.path.insert(0, "/opt/trn_rl_repo")

import numpy as np
import ml_dtypes

import concourse.bass as bass
import concourse.mybir as mybir
import concourse.tile as tile
from concourse import bacc
from concourse.bass import ts
from concourse.bass_utils import run_bass_kernel_spmd

F32 = mybir.dt.float32
F32R = mybir.dt.float32r
BF16 = mybir.dt.bfloat16
MULT = mybir.AluOpType.mult
ADD = mybir.AluOpType.add
AF = mybir.ActivationFunctionType

# ---- problem constants (hardcoded per the contract) ----
B, T, D = 4, 2048, 1024
H, HD = 8, 128
HG = 4          # heads per group
EG = HG * HD    # 512 cols per head-group
ATTN_SCALE = 0.12
RMS_EPS = 1.1920929e-07
P = 128
NT = T // P     # 16 t-blocks
ND = D // P     # 8 d-chunks
NW = T // 512   # 4 query windows
S2 = ATTN_SCALE * ATTN_SCALE

_CACHED = {}


def _rope_tables():
    af = (1.0 / 1024.0) ** np.linspace(0.0, 1.0, HD // 4, dtype=np.float32)
    af = np.concatenate([af, np.zeros(HD // 4, dtype=np.float32)])
    t = np.arange(T, dtype=np.float32)
    theta = np.einsum("i,j->ij", t, af)  # [T, 64]
    cos, sin = np.cos(theta), np.sin(theta)
    # cc: [c|c] per head; ss: [s|-s] per head -> [T, 4, 128] -> [T, 512]
    cc1 = np.concatenate([cos, cos], axis=1)            # [T,128]
    ss1 = np.concatenate([sin, -sin], axis=1)           # [T,128]
    cc = np.tile(cc1, (1, HG)).astype(np.float32)       # [T,512]
    ss = np.tile(ss1, (1, HG)).astype(np.float32)       # [T,512]
    return cc, ss


def _masks():
    # tri[tk, c] = 1 if c >= tk  (the causal edge band)
    tk = np.arange(P)[:, None]
    c = np.arange(P)[None, :]
    return (c >= tk).astype(np.float32)


def build(cfg=None):
    cfg = cfg or {}
    ptp_bufs = cfg.get("ptp_bufs", 6)
    warmup = cfg.get("warmup", 40)
    delay_rs = cfg.get("delay_rs", 2)
    nc = bacc.Bacc("TRN2", target_bir_lowering=False, debug=False)

    xT = nc.dram_tensor("xT", [D, T], BF16, kind="ExternalInput")
    wqkT = nc.dram_tensor("wqkT", [D, 2 * EG], BF16, kind="ExternalInput")
    wvT = nc.dram_tensor("wvT", [D, EG], BF16, kind="ExternalInput")
    ve = nc.dram_tensor("ve", [T, EG], BF16, kind="ExternalInput")
    cpT = nc.dram_tensor("cpT", [EG, D], BF16, kind="ExternalInput")
    cc_t = nc.dram_tensor("cc", [T, EG], BF16, kind="ExternalInput")
    ss_t = nc.dram_tensor("ss", [T, EG], BF16, kind="ExternalInput")
    mk_t = nc.dram_tensor("mk", [P, P], BF16, kind="ExternalInput")
    id_t = nc.dram_tensor("idm", [P, P], BF16, kind="ExternalInput")
    out = nc.dram_tensor("out", [T, D], F32, kind="ExternalOutput")

    xTr = xT.rearrange("(c p) t -> c p t", p=P)       # [8, 128, 2048]
    wqkr = wqkT.rearrange("(c p) e -> c p e", p=P)    # [8, 128, 1024]
    wvr = wvT.rearrange("(c p) e -> c p e", p=P)      # [8, 128, 512]
    cpr = cpT.rearrange("(c p) d -> c p d", p=P)      # [4, 128, 1024]
    ver = ve.rearrange("(i p) e -> i p e", p=P)       # [16, 128, 512]
    ccr = cc_t.rearrange("(i p) e -> i p e", p=P)
    ssr = ss_t.rearrange("(i p) e -> i p e", p=P)

    with tile.TileContext(nc) as tc:
        with (
            tc.tile_pool(name="persist", bufs=1) as pp,
            tc.tile_pool(name="consts", bufs=1) as cp,
        ):
            # persistent tensors ([P, HG, T] so one copy evicts all 4 heads)
            QTa = pp.tile([P, HG, T], BF16, tag="QTa", name="QTa")
            KTa = pp.tile([P, HG, T], BF16, tag="KTa", name="KTa")
            V = [pp.tile([P, EG], BF16, tag=f"V{i}", name=f"V{i}") for i in range(NT)]
            tri = cp.tile([P, P], BF16, tag="tri")
            SK = cp.tile([P, NT, HG], F32, tag="SK")
            ident = cp.tile([P, P], BF16, tag="ident")
            ones_col = cp.tile([P, 1], BF16, tag="ones_col")
            ones_row = cp.tile([1, P], F32, tag="ones_row")
            bias_q = cp.tile([P, 1], F32, tag="bias_q")
            bias_k = cp.tile([P, 1], F32, tag="bias_k")
            nc.vector.memset(bias_q[:], RMS_EPS / S2)
            nc.vector.memset(bias_k[:], float(RMS_EPS))
            nc.sync.dma_start(tri[:], mk_t[:, :])
            nc.sync.dma_start(ident[:], id_t[:, :])
            nc.vector.memset(ones_col[:], 1.0)
            nc.vector.memset(ones_row[:], 1.0)
            # c_proj weights prefetched during phase A
            cpt = [cp.tile([P, D], BF16, tag=f"cpt{e}", name=f"cpt{e}")
                   for e in range(HG)]
            for e in range(HG):
                nc.sync.dma_start(cpt[e][:], cpr[e])

            # ---------------- Phase A: projections, rms+rope, transposes ---
            with (
                tc.tile_pool(name="wpool", bufs=1) as wp,
                tc.tile_pool(name="xpool", bufs=3) as xp,
                tc.tile_pool(name="qkte", bufs=2) as qp,
                tc.tile_pool(name="ropetmp", bufs=2) as rp,
                tc.tile_pool(name="rrpool", bufs=5) as rrp,
                tc.tile_pool(name="tabs", bufs=2) as tp,
                tc.tile_pool(name="pA", bufs=2, space="PSUM") as pA,
                tc.tile_pool(name="pT", bufs=1, space="PSUM") as pT,
            ):
                xtis = {}

                def fetch_x(i):
                    if i < NT:
                        xi = xp.tile([P, ND, P], BF16, tag="xt", name="xt")
                        nc.sync.dma_start(
                            xi[:], xTr[:, :, ts(i, P)].rearrange("c p t -> p c t"))
                        xtis[i] = xi

                fetch_x(0)
                fetch_x(1)
                fetch_x(2)
                wqk = [wp.tile([P, 2 * EG], BF16, tag=f"wqk{c}", name=f"wqk{c}") for c in range(ND)]
                wv = [wp.tile([P, EG], BF16, tag=f"wv{c}", name=f"wv{c}") for c in range(ND)]
                for c in range(ND):
                    nc.sync.dma_start(wqk[c][:], wqkr[c])
                    nc.sync.dma_start(wv[c][:], wvr[c])

                if warmup:
                    wt = wp.tile([P, EG], BF16, tag="warmsrc", name="warmsrc")
                    nc.vector.memset(wt[:], 0.0)
                    for wi in range(warmup):
                        pw = pA.tile([P, EG], F32, tag="psq", name="warm")
                        nc.tensor.matmul(pw[0:1, :], ones_col[:], wt[:],
                                         start=True, stop=True)

                pendQ = []   # (fin, tsl) delayed 1 block
                pendK = []   # (rr_k, tsl) delayed 2 blocks

                def emit_q_transposes(rr_q, dg, tsl):
                    # regular matmul with moving = diag(rsc_h): computes
                    # rr^T @ diag(rsc) — transpose + per-token rms scale in one.
                    ptr = pT.tile([P, HG, P], F32, tag="ptrq", name="ptrq")
                    for h in range(HG):
                        nc.tensor.matmul(ptr[:, h, :], rr_q[:, ts(h, HD)],
                                         dg[:, h, :], start=True, stop=True)
                    nc.scalar.copy(QTa[:, :, tsl], ptr[:])

                def emit_k_transposes(rr_k, tsl):
                    ptr = pT.tile([P, HG, P], BF16, tag="ptrk", name="ptrk")
                    for h in range(HG):
                        nc.tensor.transpose(ptr[:, h, :], rr_k[:, ts(h, HD)],
                                            ident[:])
                    nc.scalar.copy(KTa[:, :, tsl], ptr[:])

                for i in range(NT):
                    tsl = ts(i, P)
                    xti = xtis.pop(i)

                    psq = pA.tile([P, EG], F32, tag="psq")
                    psk = pA.tile([P, EG], F32, tag="psk")
                    psv = pA.tile([P, EG], F32, tag="psv")
                    fetch_x(i + 3)
                    for c in range(ND):
                        nc.tensor.matmul(psq[:], xti[:, c, :], wqk[c][:, 0:EG],
                                         start=(c == 0), stop=(c == ND - 1))
                        nc.tensor.matmul(psk[:], xti[:, c, :], wqk[c][:, EG:2 * EG],
                                         start=(c == 0), stop=(c == ND - 1))
                        nc.tensor.matmul(psv[:], xti[:, c, :], wv[c][:],
                                         start=(c == 0), stop=(c == ND - 1))

                    # --- evict q/k early (frees PSUM for the next blocks) ----
                    qte = qp.tile([P, 2 * EG], BF16, tag="qte")
                    nc.scalar.copy(qte[:, EG:2 * EG], psk[:])
                    nc.scalar.copy(qte[:, 0:EG], psq[:])

                    cct = tp.tile([P, EG], BF16, tag="cct")
                    sst = tp.tile([P, EG], BF16, tag="sst")
                    nc.sync.dma_start(cct[:], ccr[i])
                    nc.sync.dma_start(sst[:], ssr[i])
                    s4 = sst[:].rearrange("p (h s e) -> p h s e", h=HG, s=2)

                    def rope_side(eng, src_ap, tag):
                        # t1 and the final add are contiguous [P, EG] ops; only
                        # the half-swap mults need the strided 4D view.
                        x4 = src_ap.rearrange("p (h s e) -> p h s e", h=HG, s=2)
                        t1 = rp.tile([P, EG], BF16, tag=f"t1_{tag}",
                                     name=f"t1_{tag}")
                        t2 = rp.tile([P, HG, 2, 64], BF16, tag=f"t2_{tag}",
                                     name=f"t2_{tag}")
                        eng.tensor_tensor(t1[:], src_ap, cct[:], op=MULT)
                        eng.tensor_tensor(t2[:, :, 0, :], x4[:, :, 1, :],
                                          s4[:, :, 0, :], op=MULT)
                        eng.tensor_tensor(t2[:, :, 1, :], x4[:, :, 0, :],
                                          s4[:, :, 1, :], op=MULT)
                        rr = rrp.tile([P, EG], BF16, tag=f"rr_{tag}",
                                      name=f"rr_{tag}")
                        t2f = t2[:].rearrange("p h s e -> p (h s e)")
                        eng.tensor_tensor(rr[:], t1[:], t2f, op=ADD)
                        return rr

                    # last blocks' k-rope on DVE: its chain gates phase B start
                    k_eng = nc.vector if i >= NT - 2 else nc.gpsimd
                    rr_k = rope_side(k_eng, qte[:, EG:2 * EG], "k")

                    # --- v = (l0*wv)x + (l1*ve)  (lambdas folded on host) ---
                    vet = tp.tile([P, EG], BF16, tag="vet")
                    nc.sync.dma_start(vet[:], ver[i])
                    nc.vector.tensor_tensor(V[i][:], psv[:], vet[:], op=ADD)

                    # --- q rms sumsq on DVE, then rope, then per-head scale --
                    ssq = rp.tile([P, 8], F32, tag="ssq")
                    sq_scr = rp.tile([P, P], F32, tag="sq_scr")
                    for h in range(4):
                        nc.vector.scalar_tensor_tensor(
                            sq_scr[:], qte[:, ts(h, HD)], 1.0, qte[:, ts(h, HD)],
                            op0=MULT, op1=MULT, accum_out=ssq[:, h:h + 1])
                    # scales: q gets 0.12 folded in; k scale is folded into the
                    # phase-B exp (per-partition scale), so only store recip.
                    sc = rp.tile([P, 8], F32, tag="sc")
                    nc.scalar.activation(sc[:, 0:4], ssq[:, 0:4], AF.Sqrt,
                                         scale=1.0 / (HD * S2), bias=bias_q[:])
                    rsc = rp.tile([P, 4], F32, tag="rsc")
                    nc.vector.reciprocal(rsc[:], sc[:, 0:4])
                    # diag(rsc_h) tiles for the scaled q-transposes; depends
                    # only on rsc so it overlaps the rope chain.
                    dg = rrp.tile([P, HG, P], BF16, tag="dg", name="dg")
                    for h in range(HG):
                        nc.vector.tensor_scalar(dg[:, h, :], ident[:],
                                                rsc[:, h:h + 1], None, op0=MULT)
                    rr_q = rope_side(nc.vector, qte[:, 0:EG], "q")

                    # --- k rms sumsq (only feeds phase-B exp scale) ----------
                    for h in range(4, 8):
                        nc.vector.scalar_tensor_tensor(
                            sq_scr[:], qte[:, ts(h, HD)], 1.0, qte[:, ts(h, HD)],
                            op0=MULT, op1=MULT, accum_out=ssq[:, h:h + 1])
                    nc.scalar.activation(sc[:, 4:8], ssq[:, 4:8], AF.Sqrt,
                                         scale=1.0 / HD, bias=bias_k[:])
                    nc.vector.reciprocal(SK[:, i, :], sc[:, 4:8])

                    # transposes of PREVIOUS blocks go after this block's
                    # projections in the PE queue (hides the elementwise
                    # chain); k is delayed 2 blocks (gpsimd rope is slow).
                    pendQ.append((rr_q, dg, tsl))
                    pendK.append((rr_k, tsl))
                    if len(pendQ) > 2:
                        emit_q_transposes(*pendQ.pop(0))
                    if len(pendK) > 3:
                        emit_k_transposes(*pendK.pop(0))
                for a in pendQ:
                    emit_q_transposes(*a)
                for a in pendK:
                    emit_k_transposes(*a)

            # ---------------- Phase B: attention + c_proj -------------------
            with (
                tc.tile_pool(name="ptpool", bufs=ptp_bufs) as ptp,
                tc.tile_pool(name="ypool", bufs=2) as yp,
                tc.tile_pool(name="rpool", bufs=2) as rpl,
                tc.tile_pool(name="opool", bufs=3) as op_,
                tc.tile_pool(name="pS", bufs=2, space="PSUM") as pS,
                tc.tile_pool(name="pY", bufs=2, space="PSUM") as pY,
                tc.tile_pool(name="pR", bufs=1, space="PSUM") as pR,
                tc.tile_pool(name="pO", bufs=2, space="PSUM") as pO,
            ):
                Ywin = {}       # w -> list of normalized Y tiles
                pending = None  # (w, h, ps_y, ps_r) awaiting normalization

                def emit_norm(pw, ph, ps_y, ps_r):
                    rro = rpl.tile([1, 512], F32, tag="rro", name="rro")
                    nc.vector.reciprocal_approx_fast(rro[:], ps_r[:])
                    bb = rpl.tile([P, 512], F32, tag="bb", name="bb")
                    rrow = rpl.tile([1, 512], F32R, tag="rrow", name="rrow")
                    nc.scalar.copy(rrow[:], rro[:])
                    ps_b = pR.tile([P, 512], F32, tag="ps_b", name="ps_b")
                    nc.tensor.matmul(ps_b[:], ones_row[:].bitcast(F32R),
                                     rrow[:], start=True, stop=True)
                    nc.vector.tensor_copy(bb[:], ps_b[:])
                    yh = yp.tile([P, 512], BF16, tag=f"y{ph}", name=f"y{ph}")
                    nc.vector.tensor_tensor(yh[:], ps_y[:], bb[:], op=MULT)
                    Ywin.setdefault(pw, []).append(yh)

                def emit_cproj(pw):
                    Y = Ywin.pop(pw)
                    for tb in range(4):
                        for db in range(2):
                            ps_o = pO.tile([P, 512], F32, tag="ps_o", name="ps_o")
                            for h in range(HG):
                                nc.tensor.matmul(ps_o[:], Y[h][:, ts(tb, P)],
                                                 cpt[h][:, ts(db, 512)],
                                                 start=(h == 0), stop=(h == HG - 1))
                            oc = op_.tile([P, 512], F32, tag="oc", name="oc")
                            nc.vector.tensor_copy(oc[:], ps_o[:])
                            nc.sync.dma_start(
                                out[pw * 512 + tb * P:pw * 512 + (tb + 1) * P,
                                    ts(db, 512)], oc[:])

                for w in range(NW):
                    wsl = ts(w, 512)
                    for h in range(HG):
                        nch = 4 * (w + 1)
                        ps_y = pY.tile([P, 512], F32, tag="ps_y", name="ps_y")
                        ps_r = pR.tile([1, 512], F32, tag="ps_r", name="ps_r")
                        rs_q = []

                        def v0_of(jj):
                            return max(jj - (nch - 4), 0) * P

                        def emit_pv(jj):
                            vv = v0_of(jj)
                            nc.tensor.matmul(ps_y[:, vv:512],
                                             V[jj][:, ts(h, HD)],
                                             rs_q[jj][:, vv:512],
                                             start=(jj == 0),
                                             stop=(jj == nch - 1))

                        # pv trails the score/exp stream by delay_rs j-steps
                        # so the PE never waits on the ACT exp of the same j.
                        # Row sums: pt pairs are pre-added on DVE, halving the
                        # PE row-sum matmuls (masked prefixes are zeroed).
                        prs = []

                        def emit_pair(m):
                            pa = rpl.tile([P, 512], BF16, tag="pa", name="pa")
                            va = v0_of(2 * m)
                            if va > 0:
                                nc.vector.memset(pa[:, 0:va], 0.0)
                            nc.vector.tensor_tensor(
                                pa[:, va:512], rs_q[2 * m][:, va:512],
                                rs_q[2 * m + 1][:, va:512], op=ADD)
                            prs.append(pa)

                        def emit_prs(m):
                            va = v0_of(2 * m)
                            nc.tensor.matmul(ps_r[:, va:512], ones_col[:],
                                             prs[m][:, va:512], start=(m == 0),
                                             stop=(m == nch // 2 - 1))

                        for j in range(nch):
                            v0 = v0_of(j)
                            ps_s = pS.tile([P, 512], F32, tag="ps_s", name="ps_s")
                            nc.tensor.matmul(ps_s[:, v0:512], KTa[:, h, ts(j, P)],
                                             QTa[:, h, w * 512 + v0:(w + 1) * 512],
                                             start=True, stop=True)
                            pt = ptp.tile([P, 512], BF16, tag="pt", name="pt")
                            if v0 > 0:
                                nc.vector.memset(pt[:, 0:v0], 0.0)
                            nc.scalar.activation(pt[:, v0:512], ps_s[:, v0:512],
                                                 AF.Exp, scale=SK[:, j, h:h + 1])
                            if j >= nch - 4:  # mask the 128-wide triangle band
                                nc.gpsimd.tensor_tensor(
                                    pt[:, v0:v0 + P], pt[:, v0:v0 + P], tri[:],
                                    op=MULT)
                            rs_q.append(pt)
                            if j % 2 == 1:
                                emit_pair(j // 2)
                                if j // 2 >= 1:
                                    emit_prs(j // 2 - 1)
                            if j >= delay_rs:
                                emit_pv(j - delay_rs)
                        for jj in range(max(nch - delay_rs, 0), nch):
                            emit_pv(jj)
                        emit_prs(nch // 2 - 1)
                        # normalize the PREVIOUS head now that this head's
                        # matmuls are queued (hides the recip->bcast latency)
                        if pending is not None:
                            emit_norm(*pending)
                            if pending[1] == HG - 1:
                                emit_cproj(pending[0])
                        pending = (w, h, ps_y, ps_r)
                emit_norm(*pending)
                emit_cproj(pending[0])
    nc.compile()
    return nc


def _get_nc():
    if "nc" not in _CACHED:
        _CACHED["nc"] = build()
    return _CACHED["nc"]


def _try_install_profile_shim():
    try:
        import contextlib
        import ctypes
        import types

        if "antenv.axon_hooks" in sys.modules:
            return
        so_path = "/opt/axon/libaxon_pjrt.so"
        lib = ctypes.CDLL(so_path)
        if not hasattr(lib, "axon_start_nrt_profile"):
            return
        lib.axon_start_nrt_profile.argtypes = [ctypes.POINTER(ctypes.c_int64),
                                               ctypes.c_size_t]
        lib.axon_start_nrt_profile.restype = ctypes.c_int64
        lib.axon_stop_nrt_profile.argtypes = [ctypes.c_char_p]
        lib.axon_stop_nrt_profile.restype = ctypes.c_int64

        @contextlib.contextmanager
        def _hook(output_dir, device_ids):
            import jax

            jax.devices()
            if device_ids:
                ids = (ctypes.c_int64 * len(device_ids))(*device_ids)
                rc = lib.axon_start_nrt_profile(ids, len(device_ids))
            else:
                rc = lib.axon_start_nrt_profile(None, 0)
            if rc != 0:
                raise RuntimeError(f"axon_start_nrt_profile rc={rc}")
            try:
                yield
            finally:
                lib.axon_stop_nrt_profile(str(output_dir).encode())

        mod = types.ModuleType("antenv.axon_hooks")
        mod.set_axon_ntff_profile_hook = lambda h: None
        mod.get_axon_ntff_profile_hook = lambda: _hook
        import antenv

        antenv.axon_hooks = mod
        sys.modules["antenv.axon_hooks"] = mod
    except Exception:
        pass


LAST_EXEC_TIME_NS = None


def kernel(x, ve, sa_lambdas, qkv_w, c_proj_weight):
    global LAST_EXEC_TIME_NS
    x = np.asarray(x, dtype=np.float32)
    ve = np.asarray(ve, dtype=np.float32)
    sa_lambdas = np.asarray(sa_lambdas, dtype=np.float32)
    qkv_w = np.asarray(qkv_w, dtype=np.float32)
    c_proj_weight = np.asarray(c_proj_weight, dtype=np.float32)

    def tobf(a):
        return np.ascontiguousarray(a).astype(ml_dtypes.bfloat16)

    cc, ss = _rope_tables()
    mk = _masks()
    idm = np.eye(P, dtype=np.float32)
    l0, l1 = float(sa_lambdas[0]), float(sa_lambdas[1])

    in_maps = []
    for c in range(8):
        b, g = c // 2, c % 2
        gs, ge = g * EG, (g + 1) * EG
        wq = qkv_w[0, gs:ge, :]           # [512, 1024]
        wk = qkv_w[1, gs:ge, :]
        wv = qkv_w[2, gs:ge, :] * l0      # lambda0 folded
        in_maps.append({
            "xT": tobf(x[b].T),                                       # [D, T]
            "wqkT": tobf(np.concatenate([wq, wk], axis=0).T),         # [D, 1024]
            "wvT": tobf(wv.T),                                        # [D, 512]
            "ve": tobf(
                ve[b].reshape(T, H, HD)[:, g * HG:(g + 1) * HG, :]
                .reshape(T, EG) * l1),                                # [T, 512]
            "cpT": tobf(c_proj_weight[:, gs:ge].T),                   # [512, D]
            "cc": tobf(cc), "ss": tobf(ss), "mk": tobf(mk),
            "idm": tobf(idm),
        })

    _try_install_profile_shim()
    nc = _get_nc()
    res = run_bass_kernel_spmd(nc, in_maps, core_ids=list(range(8)), trace=True)
    LAST_EXEC_TIME_NS = res.exec_time_ns

    outs = [res.results[c]["out"] for c in range(8)]
    full = np.stack([outs[2 * b] + outs[2 * b + 1] for b in range(B)], axis=0)
    return full.astype(np.float32)
